# revision 4
# baseline (speedup 1.0000x reference)
"""Multi-head attention (B=2, S=2048, D=1024, H=16) on 8 trn2 NeuronCores.

Sharding: core c handles heads {2c, 2c+1} for BOTH batches (tensor parallel by
head). Token axis is flattened b-major: T = B*S = 4096.
 - Q/K/V projections computed per-core for its 2 heads (column-sharded weights,
   host-transposed to [D, .] so contraction sits on partitions).
 - Attention in transposed orientation: scoresT[j,i] tiles on PE, exp on ACT
   (scale=1/8 folded in), causal masking via gpsimd affine_select on the exp
   output, PV with ones-augmented V so the softmax denominator falls out of the
   same accumulation (row 64 of the PV psum). Normalization: DVE reciprocal of
   the denom row, broadcast across 64 partitions with a K=1 fp32 matmul, DVE mul.
 - Output projection: 8-rank AllToAll reshards concat^T from head-sharded to
   token-sharded; each core then computes 512 token rows of out = concat @ Wo^T.
All matmuls bf16 with fp32 PSUM accumulation. Host pre-transposes/casts inputs.
"""

import sys

sys.path.insert(0, "/opt/trn_rl_repo")

import numpy as np
import ml_dtypes

import concourse.bass as bass
import concourse.mybir as mybir
import concourse.tile as tile
from concourse import bacc
from concourse import bass_utils

B, S, D, H = 2, 2048, 1024, 16
DK = D // H              # 64
N_CORES = 8
HPC = H // N_CORES       # heads per core (2)
EPC = HPC * DK           # 128 projected cols per core
T = B * S                # 4096 flattened tokens
TOK = T // N_CORES       # 512 output tokens per core
IT = 512                 # i (query) tile
JT = 128                 # j (key) tile
NIT = S // IT            # 4 i-tiles per batch
NJT = S // JT            # 16 j-tiles per batch
NST = T // IT            # 8 projection token tiles
ND = D // 128            # 8 contraction tiles
VST = DK + 1             # 65: V block width with ones column

bf16 = mybir.dt.bfloat16
f32 = mybir.dt.float32
BF = ml_dtypes.bfloat16

_CACHE: dict = {}


def _build(mode: str):
    """mode: 'causal' | 'none' | 'generic'"""
    nc = bacc.Bacc("TRN2", target_bir_lowering=False, debug=False,
                   enable_asserts=False, num_devices=N_CORES)

    xq = nc.dram_tensor("xq", [D, T], bf16, kind="ExternalInput")
    xk = nc.dram_tensor("xk", [D, T], bf16, kind="ExternalInput")
    xv = nc.dram_tensor("xv", [D, T], bf16, kind="ExternalInput")
    wq = nc.dram_tensor("wq", [D, EPC], bf16, kind="ExternalInput")
    wk = nc.dram_tensor("wk", [D, EPC], bf16, kind="ExternalInput")
    wv = nc.dram_tensor("wv", [D, EPC], bf16, kind="ExternalInput")
    wo = nc.dram_tensor("wo", [D, D], bf16, kind="ExternalInput")
    if mode == "generic":
        bias = nc.dram_tensor("bias", [S, S], bf16, kind="ExternalInput")
    out = nc.dram_tensor("out", [TOK, D], f32, kind="ExternalOutput")

    Exp = mybir.ActivationFunctionType.Exp
    rg = [list(range(N_CORES))]

    with tile.TileContext(nc) as tc:
        with (
            tc.tile_pool(name="consts", bufs=1) as consts,
            tc.tile_pool(name="persist", bufs=1) as persist,
            tc.tile_pool(name="dram", bufs=1, space="DRAM") as dram,
        ):
            # --- persistent SBUF tensors ---
            wq_sb = [consts.tile([128, EPC], bf16, tag=f"wq{d}", name=f"wq{d}")
                     for d in range(ND)]
            wk_sb = [consts.tile([128, EPC], bf16, tag=f"wk{d}", name=f"wk{d}")
                     for d in range(ND)]
            wv_sb = [consts.tile([128, EPC], bf16, tag=f"wv{d}", name=f"wv{d}")
                     for d in range(ND)]
            wo_sb = [consts.tile([128, D], bf16, tag=f"wo{d}", name=f"wo{d}")
                     for d in range(ND)]
            ones1 = consts.tile([1, DK], f32, tag="ones1")
            nc.vector.memset(ones1[:], 1.0)

            QT = persist.tile([128, T], bf16, tag="QT")
            KT = persist.tile([128, T], bf16, tag="KT")
            CT = persist.tile([128, T], bf16, tag="CT")
            NJ_ALL = T // JT     # 32 j-tiles across both batches
            V_all = persist.tile([128, NJ_ALL * HPC * VST], bf16, tag="V_all")
            v4 = V_all[:].rearrange("p (t h c) -> p (t h) c",
                                    t=NJ_ALL, h=HPC, c=VST)
            nc.vector.memset(v4[:, :, DK:DK + 1], 1.0)

            for d in range(ND):
                nc.sync.dma_start(wq_sb[d][:], wq.ap()[128 * d:128 * (d + 1), :])
                nc.sync.dma_start(wk_sb[d][:], wk.ap()[128 * d:128 * (d + 1), :])
                nc.sync.dma_start(wv_sb[d][:], wv.ap()[128 * d:128 * (d + 1), :])
                nc.sync.dma_start(wo_sb[d][:], wo.ap()[128 * d:128 * (d + 1), :])

            # ================= Phase 1: projections =================
            with (
                tc.tile_pool(name="xqk", bufs=16) as xqk,
                tc.tile_pool(name="xvp", bufs=24) as xvp,
                tc.tile_pool(name="psA", bufs=1, space="PSUM") as psA,
                tc.tile_pool(name="psV", bufs=1, space="PSUM") as psV,
            ):
                for wsb, xdram, dest in ((wq_sb, xq, QT), (wk_sb, xk, KT)):
                    for st in range(NST):
                        xts = []
                        for d in range(ND):
                            xt = xqk.tile([128, IT], bf16, tag="x", name="xt")
                            nc.sync.dma_start(
                                xt[:],
                                xdram.ap()[128 * d:128 * (d + 1),
                                           IT * st:IT * (st + 1)])
                            xts.append(xt)
                        ps = psA.tile([128, IT], f32, tag="proj", bufs=2,
                                      name="ps")
                        for d in range(ND):
                            nc.tensor.matmul(ps[:], wsb[d][:], xts[d][:],
                                             start=(d == 0), stop=(d == ND - 1))
                        nc.vector.tensor_copy(
                            dest[:, IT * st:IT * (st + 1)], ps[:])

                # V: out[j, c] for the 2 local heads; lhsT = xvT tile
                for jt in range(NJ_ALL):
                    xvs = []
                    for d in range(ND):
                        xt = xvp.tile([128, JT], bf16, tag="xv", name="xvt")
                        nc.sync.dma_start(
                            xt[:],
                            xv.ap()[128 * d:128 * (d + 1),
                                    JT * jt:JT * (jt + 1)])
                        xvs.append(xt)
                    psv = psV.tile([128, EPC], f32, tag="vproj", bufs=2,
                                   name="psv")
                    for d in range(ND):
                        nc.tensor.matmul(psv[:], xvs[d][:], wv_sb[d][:],
                                         start=(d == 0), stop=(d == ND - 1))
                    dst = V_all[:, VST * HPC * jt:VST * HPC * (jt + 1)]
                    nc.vector.tensor_copy(
                        dst.rearrange("p (h c) -> p h c", h=HPC, c=VST)[:, :, 0:DK],
                        psv[:].rearrange("p (h c) -> p h c", h=HPC, c=DK))

            # ================= Phase 2: attention =================
            with (
                tc.tile_pool(name="psP", bufs=1, space="PSUM") as psP,
                tc.tile_pool(name="psO", bufs=1, space="PSUM") as psO,
                tc.tile_pool(name="psB", bufs=1, space="PSUM") as psB,
                tc.tile_pool(name="sbE", bufs=1) as sbE,
                tc.tile_pool(name="sbR", bufs=1) as sbR,
                tc.tile_pool(name="biasp", bufs=4) as biasp,
            ):
                for hl in range(HPC):
                    pb = 64 * hl
                    for b in range(B):
                        tok0 = S * b
                        for it in range(NIT):
                            il0 = IT * it            # batch-local i offset
                            i0 = tok0 + il0
                            njt = (il0 + IT) // JT if mode == "causal" else NJT
                            po = psO.tile([VST, IT], f32, tag="po", bufs=2,
                                          name="po")
                            for jl in range(njt):
                                jabs = NJT * b + jl
                                pt = psP.tile([128, IT], f32, tag="pt", bufs=3,
                                              name="pt")
                                nc.tensor.matmul(
                                    pt[:],
                                    KT[pb:pb + DK, JT * jabs:JT * (jabs + 1)],
                                    QT[pb:pb + DK, i0:i0 + IT],
                                    start=True, stop=True)
                                if mode == "generic":
                                    bs = biasp.tile([128, IT], bf16, tag="bias",
                                                    name="bs")
                                    nc.sync.dma_start(
                                        bs[:],
                                        bias.ap()[JT * jl:JT * (jl + 1),
                                                  il0:il0 + IT])
                                    nc.vector.tensor_add(pt[:], pt[:], bs[:])
                                ex = sbE.tile([128, IT], bf16, tag="expp",
                                              bufs=6, name="ex")
                                nc.scalar.activation(ex[:], pt[:], Exp,
                                                     scale=0.125)
                                if mode == "causal" and JT * jl > il0 - 1:
                                    # keep iff (jl0 + p) <= (il0 + f)
                                    nc.gpsimd.affine_select(
                                        out=ex[:], in_=ex[:],
                                        compare_op=mybir.AluOpType.is_ge,
                                        fill=0.0,
                                        base=il0 - JT * jl,
                                        pattern=[[1, IT]],
                                        channel_multiplier=-1)
                                voff = VST * (HPC * jabs + hl)
                                nc.tensor.matmul(po[:],
                                                 V_all[:, voff:voff + VST],
                                                 ex[:],
                                                 start=(jl == 0),
                                                 stop=(jl == njt - 1))
                            rec = sbR.tile([1, IT], f32, tag="rec", bufs=2,
                                           name="rec")
                            nc.vector.reciprocal(rec[:], po[DK:DK + 1, :])
                            pbc = psB.tile([DK, IT], f32, tag="pb", bufs=2,
                                           name="pbc")
                            nc.tensor.matmul(pbc[:], ones1[:], rec[:],
                                             start=True, stop=True)
                            pbs = sbR.tile([DK, IT], f32, tag="pbs", bufs=2,
                                           name="pbs")
                            nc.scalar.copy(pbs[:], pbc[:])
                            nc.vector.tensor_mul(CT[pb:pb + DK, i0:i0 + IT],
                                                 po[0:DK, :], pbs[:])

            # ============ Phase 3: A2A reshard + output projection ============
            bounceA = dram.tile([N_CORES * 128, TOK], bf16)
            bounceB = dram.tile([N_CORES * 128, TOK], bf16)
            for r in range(N_CORES):
                nc.sync.dma_start(bounceA[128 * r:128 * (r + 1), :],
                                  CT[:, TOK * r:TOK * (r + 1)])
            nc.gpsimd.collective_compute(
                "AllToAll", mybir.AluOpType.bypass, replica_groups=rg,
                ins=[bounceA[:]], outs=[bounceB[:]])

            with (
                tc.tile_pool(name="ctp", bufs=20) as ctp,
                tc.tile_pool(name="psF", bufs=1, space="PSUM") as psF,
                tc.tile_pool(name="sbF", bufs=1) as sbF,
            ):
                for tt in range(TOK // 128):
                    cts = []
                    for d in range(ND):
                        ct = ctp.tile([128, 128], bf16, tag="ct", name="ct")
                        nc.sync.dma_start(
                            ct[:],
                            bounceB[128 * d:128 * (d + 1),
                                    128 * tt:128 * (tt + 1)])
                        cts.append(ct)
                    for eh in range(2):
                        pf = psF.tile([128, 512], f32, tag="pf", bufs=2,
                                      name="pf")
                        for d in range(ND):
                            nc.tensor.matmul(
                                pf[:], cts[d][:],
                                wo_sb[d][:, 512 * eh:512 * (eh + 1)],
                                start=(d == 0), stop=(d == ND - 1))
                        of = sbF.tile([128, 512], f32, tag="of", bufs=3,
                                      name="of")
                        nc.vector.tensor_copy(of[:], pf[:])
                        nc.sync.dma_start(
                            out.ap()[128 * tt:128 * (tt + 1),
                                     512 * eh:512 * (eh + 1)], of[:])

    nc.compile()
    return nc


def _prep(inputs, mode):
    query = np.asarray(inputs["query"], np.float32)
    key = np.asarray(inputs["key"], np.float32)
    value = np.asarray(inputs["value"], np.float32)
    Wq = np.asarray(inputs["Wq"], np.float32)
    Wk = np.asarray(inputs["Wk"], np.float32)
    Wv = np.asarray(inputs["Wv"], np.float32)
    Wo = np.asarray(inputs["Wo"], np.float32)

    xqT = np.ascontiguousarray(query.reshape(T, D).T).astype(BF)
    xkT = np.ascontiguousarray(key.reshape(T, D).T).astype(BF)
    xvT = np.ascontiguousarray(value.reshape(T, D).T).astype(BF)
    woT = np.ascontiguousarray(Wo.T).astype(BF)
    wqT = [np.ascontiguousarray(Wq[EPC * c:EPC * (c + 1), :].T).astype(BF)
           for c in range(N_CORES)]
    wkT = [np.ascontiguousarray(Wk[EPC * c:EPC * (c + 1), :].T).astype(BF)
           for c in range(N_CORES)]
    wvT = [np.ascontiguousarray(Wv[EPC * c:EPC * (c + 1), :].T).astype(BF)
           for c in range(N_CORES)]

    biasT = None
    if mode == "generic":
        m2 = np.asarray(inputs["mask"])[0, 0]
        biasT = np.ascontiguousarray(
            np.where(m2.T == 0, np.float32(-1e9), np.float32(0.0))).astype(BF)

    in_maps = []
    for c in range(N_CORES):
        m = {"xq": xqT, "xk": xkT, "xv": xvT,
             "wq": wqT[c], "wk": wkT[c], "wv": wvT[c], "wo": woT}
        if biasT is not None:
            m["bias"] = biasT
        in_maps.append(m)
    return in_maps


def _mask_mode(mask):
    m2 = np.asarray(mask)[0, 0]
    if (m2 == 1).all():
        return "none"
    if np.array_equal(m2 != 0, np.tril(np.ones(m2.shape, dtype=bool))):
        return "causal"
    return "generic"


def kernel(**inputs) -> np.ndarray:
    mode = _mask_mode(inputs["mask"])
    if mode not in _CACHE:
        _CACHE[mode] = _build(mode)
    nc = _CACHE[mode]
    in_maps = _prep(inputs, mode)
    res = bass_utils.run_bass_kernel_spmd(nc, in_maps,
                                          core_ids=list(range(N_CORES)))
    out = np.empty((T, D), np.float32)
    for c in range(N_CORES):
        out[TOK * c:TOK * (c + 1), :] = res.results[c]["out"]
    return out.reshape(B, S, D)


if __name__ == "__main__":
    rng = np.random.default_rng(0)
    inputs = {
        "query": rng.standard_normal((B, S, D)).astype(np.float32),
        "key": rng.standard_normal((B, S, D)).astype(np.float32),
        "value": rng.standard_normal((B, S, D)).astype(np.float32),
        "mask": np.tril(np.ones((S, S), np.int32))[None, None],
        "Wq": (rng.standard_normal((D, D)) / 32).astype(np.float32),
        "Wk": (rng.standard_normal((D, D)) / 32).astype(np.float32),
        "Wv": (rng.standard_normal((D, D)) / 32).astype(np.float32),
        "Wo": (rng.standard_normal((D, D)) / 32).astype(np.float32),
    }
    got = kernel(**inputs)
    print("kernel ran, out shape", got.shape, "finite:", np.isfinite(got).all())


# revision 13
# speedup vs baseline: 22980.9424x; 22980.9424x over previous
"""Multi-head attention (B=2, S=2048, D=1024, H=16) on 8 trn2 NeuronCores.

Sharding: core c handles heads {2c, 2c+1} for BOTH batches (tensor parallel by
head). Token axis is flattened b-major: T = B*S = 4096.
 - Q/K/V projections computed per-core for its 2 heads (column-sharded weights,
   host-transposed to [D, .] so contraction sits on partitions).
 - Attention in transposed orientation: scoresT[j,i] tiles on PE, exp on ACT
   (scale=1/8 folded in), causal masking via gpsimd affine_select on the exp
   output, PV with ones-augmented V so the softmax denominator falls out of the
   same accumulation (row 64 of the PV psum). Normalization: DVE reciprocal of
   the denom row, broadcast across 64 partitions with a K=1 fp32 matmul, DVE mul.
 - Output projection: 8-rank AllToAll reshards concat^T from head-sharded to
   token-sharded; each core then computes 512 token rows of out = concat @ Wo^T.
All matmuls bf16 with fp32 PSUM accumulation. Host pre-transposes/casts inputs.
"""

import sys

sys.path.insert(0, "/opt/trn_rl_repo")

import numpy as np
import ml_dtypes

import concourse.bass as bass
import concourse.mybir as mybir
import concourse.tile as tile
from concourse import bacc
from concourse import bass_utils

B, S, D, H = 2, 2048, 1024, 16
DK = D // H              # 64
N_CORES = 8
HPC = H // N_CORES       # heads per core (2)
EPC = HPC * DK           # 128 projected cols per core
T = B * S                # 4096 flattened tokens
TOK = T // N_CORES       # 512 output tokens per core
IT = 512                 # i (query) tile
JT = 128                 # j (key) tile
NIT = S // IT            # 4 i-tiles per batch
NJT = S // JT            # 16 j-tiles per batch
NST = T // IT            # 8 projection token tiles
ND = D // 128            # 8 contraction tiles
VST = DK + 1             # 65: V block width with ones column

bf16 = mybir.dt.bfloat16
f32 = mybir.dt.float32
f16 = mybir.dt.float16
BF = ml_dtypes.bfloat16

_CACHE: dict = {}


def _store_junk(nc, tc, out):
    import concourse.mybir as _mb
    with tc.tile_pool(name="junk", bufs=1) as jp:
        jt_ = jp.tile([128, D], _mb.dt.float32, name="junk")
        nc.vector.memset(jt_[:], 0.0)
        for tt in range(TOK // 128):
            nc.sync.dma_start(out.ap()[128 * tt:128 * (tt + 1), :], jt_[:])


def _build(mode: str, repeats: int = 1, upto: str = "full"):
    """mode: 'causal' | 'none' | 'generic'. repeats>1 builds a timing variant
    that executes the whole body N times in one NEFF. upto: 'full' | 'p2' |
    'p1' truncates after attention / projections (timing ablation only)."""
    nc = bacc.Bacc("TRN2", target_bir_lowering=False, debug=False,
                   enable_asserts=False, num_devices=N_CORES)

    xq = nc.dram_tensor("xq", [D, T], bf16, kind="ExternalInput")
    xk = nc.dram_tensor("xk", [D, T], bf16, kind="ExternalInput")
    xv = nc.dram_tensor("xv", [D, T], bf16, kind="ExternalInput")
    wq = nc.dram_tensor("wq", [D, EPC], bf16, kind="ExternalInput")
    wk = nc.dram_tensor("wk", [D, EPC], bf16, kind="ExternalInput")
    wv = nc.dram_tensor("wv", [D, EPC], bf16, kind="ExternalInput")
    wo = nc.dram_tensor("wo", [128, D], bf16, kind="ExternalInput")
    if mode == "generic":
        bias = nc.dram_tensor("bias", [S, S], bf16, kind="ExternalInput")
    out = nc.dram_tensor("out", [T, D], f16, kind="ExternalOutput")

    Exp = mybir.ActivationFunctionType.Exp
    rg = [list(range(N_CORES))]

    with tile.TileContext(nc) as tc:
      for _rep in range(repeats):
        with (
            tc.tile_pool(name="consts", bufs=1) as consts,
            tc.tile_pool(name="persist", bufs=1) as persist,
            tc.tile_pool(name="dram", bufs=1, space="DRAM") as dram,
        ):
            # --- persistent SBUF tensors ---
            wqb = consts.tile([128, ND * EPC], bf16, tag="wqb", name="wqb")
            wkb = consts.tile([128, ND * EPC], bf16, tag="wkb", name="wkb")
            wvb = consts.tile([128, ND * EPC], bf16, tag="wvb", name="wvb")
            wob = consts.tile([128, D], bf16, tag="wob", name="wob")
            wq_sb = [wqb[:, EPC * d:EPC * (d + 1)] for d in range(ND)]
            wk_sb = [wkb[:, EPC * d:EPC * (d + 1)] for d in range(ND)]
            wv_sb = [wvb[:, EPC * d:EPC * (d + 1)] for d in range(ND)]

            QT = persist.tile([128, T], bf16, tag="QT")
            KT = persist.tile([128, T], bf16, tag="KT")
            CT = persist.tile([128, T], bf16, tag="CT")
            NJ_ALL = T // JT     # 32 j-tiles across both batches
            V_all = persist.tile([128, NJ_ALL * HPC * VST], bf16, tag="V_all")
            v4 = V_all[:].rearrange("p (t h c) -> p (t h) c",
                                    t=NJ_ALL, h=HPC, c=VST)
            nc.vector.memset(v4[:, :, DK:DK + 1], 1.0)

            nc.sync.dma_start(
                wqb[:].rearrange("p (d e) -> p d e", d=ND, e=EPC),
                wq.ap().rearrange("(d p) e -> p d e", p=128))
            nc.sync.dma_start(
                wkb[:].rearrange("p (d e) -> p d e", d=ND, e=EPC),
                wk.ap().rearrange("(d p) e -> p d e", p=128))
            nc.sync.dma_start(
                wvb[:].rearrange("p (d e) -> p d e", d=ND, e=EPC),
                wv.ap().rearrange("(d p) e -> p d e", p=128))
            nc.sync.dma_start(wob[:], wo.ap())

            # ================= Phase 1: projections =================
            with (
                tc.tile_pool(name="xbig", bufs=10) as xbig,
                tc.tile_pool(name="psA", bufs=1, space="PSUM") as psA,
            ):
                for wsb, xdram, dest in ((wq_sb, xq, QT), (wk_sb, xk, KT)):
                    # d-outer: each x d-tile load feeds 8 st matmuls, then its
                    # slot frees; the 8 st accumulators live in 8 PSUM banks.
                    pss = [psA.tile([128, IT], f32, tag="proj", bufs=NST,
                                    name=f"ps{st}") for st in range(NST)]
                    for d in range(ND):
                        xt = xbig.tile([128, T], bf16, tag="xbig", name="xt")
                        nc.sync.dma_start(xt[:],
                                          xdram.ap()[128 * d:128 * (d + 1), :])
                        for st in range(NST):
                            nc.tensor.matmul(
                                pss[st][:], wsb[d],
                                xt[:, IT * st:IT * (st + 1)],
                                start=(d == 0), stop=(d == ND - 1))
                    for st in range(NST):
                        nc.vector.tensor_copy(
                            dest[:, IT * st:IT * (st + 1)], pss[st][:])

                # V: out[j, c] for the 2 local heads; lhsT = xvT tile slices
                xvs = []
                for d in range(ND):
                    xt = xbig.tile([128, T], bf16, tag="xbig", name="xvt")
                    nc.sync.dma_start(xt[:], xv.ap()[128 * d:128 * (d + 1), :])
                    xvs.append(xt)
                for jt in range(NJ_ALL):
                    psv = psA.tile([128, EPC], f32, tag="proj", bufs=NST,
                                   name="psv")
                    for d in range(ND):
                        nc.tensor.matmul(
                            psv[:], xvs[d][:, JT * jt:JT * (jt + 1)],
                            wv_sb[d], start=(d == 0), stop=(d == ND - 1))
                    dst = V_all[:, VST * HPC * jt:VST * HPC * (jt + 1)]
                    nc.vector.tensor_copy(
                        dst.rearrange("p (h c) -> p h c", h=HPC, c=VST)[:, :, 0:DK],
                        psv[:].rearrange("p (h c) -> p h c", h=HPC, c=DK))

            # ================= Phase 2: attention =================
            if upto == "p1":
                _store_junk(nc, tc, out)
                continue
            # Two heads interleaved per j-tile: breaks the pt->exp->po latency
            # chain and puts the two K=64 matmuls on different PE row groups
            # (base partitions 0/64) so they run concurrently on the array.
            with (
                tc.tile_pool(name="psP", bufs=1, space="PSUM") as psP,
                tc.tile_pool(name="psO", bufs=1, space="PSUM") as psO,
                tc.tile_pool(name="sbE", bufs=1) as sbE,
                tc.tile_pool(name="sbR", bufs=1) as sbR,
                tc.tile_pool(name="biasp", bufs=4) as biasp,
                tc.tile_pool(name="sbF", bufs=1) as sbF,
            ):
                for b in range(B):
                    tok0 = S * b
                    for it in range(NIT):
                        il0 = IT * it            # batch-local i offset
                        i0 = tok0 + il0
                        njt = (il0 + IT) // JT if mode == "causal" else NJT
                        pos = [psO.tile([VST, IT], f32, tag="po", bufs=3,
                                        name=f"po{hl}") for hl in range(HPC)]
                        pend = []   # (jl, hl, ex) exp'd tiles awaiting PV
                        for jl in range(njt):
                            jabs = NJT * b + jl
                            j0 = JT * jl             # batch-local j offset
                            diag = mode == "causal" and j0 > il0 - 1
                            # live i-columns of this block: i >= j0 (causal)
                            off = max(0, j0 - il0) if mode == "causal" else 0
                            nl = IT - off            # live width
                            bs = None
                            if mode == "generic":
                                bs = biasp.tile([128, IT], bf16, tag="bias",
                                                name="bs")
                                nc.sync.dma_start(
                                    bs[:],
                                    bias.ap()[JT * jl:JT * (jl + 1),
                                              il0:il0 + IT])
                            for hl in range(HPC):
                                pb = 64 * hl
                                pt = psP.tile([128, IT], f32, tag="pt",
                                              bufs=5, name="pt")
                                nc.tensor.matmul(
                                    pt[:, 0:nl],
                                    KT[pb:pb + DK, JT * jabs:JT * (jabs + 1)],
                                    QT[pb:pb + DK, i0 + off:i0 + IT],
                                    start=True, stop=True)
                                if bs is not None:
                                    nc.vector.tensor_add(pt[:, 0:nl],
                                                         pt[:, 0:nl],
                                                         bs[:, off:IT])
                                ex = sbE.tile([128, IT], bf16, tag="expp",
                                              bufs=12, name="ex")
                                nc.scalar.activation(ex[:, 0:nl],
                                                     pt[:, 0:nl], Exp,
                                                     scale=0.125)
                                if diag:
                                    # triangular part lives in the first JT
                                    # live cols: keep iff j0+p <= j0+f
                                    nc.gpsimd.affine_select(
                                        out=ex[:, 0:JT], in_=ex[:, 0:JT],
                                        compare_op=mybir.AluOpType.is_ge,
                                        fill=0.0,
                                        base=0,
                                        pattern=[[1, JT]],
                                        channel_multiplier=-1)
                                pend.append((jl, hl, ex, off, nl))
                            # emit PV one j-step behind so each po matmul's
                            # exp input was produced during the previous
                            # j-step's score matmuls (keeps PE from stalling)
                            while len(pend) > 2 * HPC:
                                pjl, phl, pex, poff, pnl = pend.pop(0)
                                pjabs = NJT * b + pjl
                                voff = VST * (HPC * pjabs + phl)
                                nc.tensor.matmul(pos[phl][:, poff:IT],
                                                 V_all[:, voff:voff + VST],
                                                 pex[:, 0:pnl],
                                                 start=(pjl == 0),
                                                 stop=(pjl == njt - 1))
                        for pjl, phl, pex, poff, pnl in pend:
                            pjabs = NJT * b + pjl
                            voff = VST * (HPC * pjabs + phl)
                            nc.tensor.matmul(pos[phl][:, poff:IT],
                                             V_all[:, voff:voff + VST],
                                             pex[:, 0:pnl],
                                             start=(pjl == 0),
                                             stop=(pjl == njt - 1))
                        for hl in range(HPC):
                            pb = 64 * hl
                            rec = sbR.tile([1, IT], f32, tag="rec", bufs=2,
                                           name="rec")
                            nc.vector.reciprocal(rec[:],
                                                 pos[hl][DK:DK + 1, :])
                            pbs = sbR.tile([DK, IT], f32, tag="pbs", bufs=2,
                                           name="pbs")
                            nc.gpsimd.partition_broadcast(pbs[:], rec[:])
                            nc.vector.tensor_mul(CT[pb:pb + DK, i0:i0 + IT],
                                                 pos[hl][0:DK, :], pbs[:])
                        # inline partial output projection for this i-block:
                        # out_partial[t, e] = sum_{local d} CT[d, t] * woT[d, e]
                        of = sbF.tile([128, (IT // 128) * D], f16, tag="of",
                                      bufs=2, name="of")
                        for tt in range(IT // 128):
                            t0 = i0 + 128 * tt
                            for eh in range(2):
                                pf = psP.tile([128, IT], f32, tag="pt",
                                              bufs=5, name="pf")
                                nc.tensor.matmul(
                                    pf[:], CT[:, t0:t0 + 128],
                                    wob[:, 512 * eh:512 * (eh + 1)],
                                    start=True, stop=True)
                                nc.vector.tensor_copy(
                                    of[:, D * tt + 512 * eh:
                                       D * tt + 512 * (eh + 1)], pf[:])
                        nc.sync.dma_start(
                            out.ap()[i0:i0 + IT, :].rearrange(
                                "(tt p) e -> p tt e", p=128),
                            of[:].rearrange("p (tt e) -> p tt e",
                                            tt=IT // 128, e=D))

    nc.compile()
    return nc


def _prep(inputs, mode):
    query = np.asarray(inputs["query"], np.float32)
    key = np.asarray(inputs["key"], np.float32)
    value = np.asarray(inputs["value"], np.float32)
    Wq = np.asarray(inputs["Wq"], np.float32)
    Wk = np.asarray(inputs["Wk"], np.float32)
    Wv = np.asarray(inputs["Wv"], np.float32)
    Wo = np.asarray(inputs["Wo"], np.float32)

    xqT = np.ascontiguousarray(query.reshape(T, D).T).astype(BF)
    xkT = np.ascontiguousarray(key.reshape(T, D).T).astype(BF)
    xvT = np.ascontiguousarray(value.reshape(T, D).T).astype(BF)
    woT = np.ascontiguousarray(Wo.T).astype(BF)
    woT_loc = [np.ascontiguousarray(woT[128 * c:128 * (c + 1), :])
               for c in range(N_CORES)]
    wqT = [np.ascontiguousarray(Wq[EPC * c:EPC * (c + 1), :].T).astype(BF)
           for c in range(N_CORES)]
    wkT = [np.ascontiguousarray(Wk[EPC * c:EPC * (c + 1), :].T).astype(BF)
           for c in range(N_CORES)]
    wvT = [np.ascontiguousarray(Wv[EPC * c:EPC * (c + 1), :].T).astype(BF)
           for c in range(N_CORES)]

    biasT = None
    if mode == "generic":
        m2 = np.asarray(inputs["mask"])[0, 0]
        biasT = np.ascontiguousarray(
            np.where(m2.T == 0, np.float32(-1e9), np.float32(0.0))).astype(BF)

    in_maps = []
    for c in range(N_CORES):
        m = {"xq": xqT, "xk": xkT, "xv": xvT,
             "wq": wqT[c], "wk": wkT[c], "wv": wvT[c], "wo": woT_loc[c]}
        if biasT is not None:
            m["bias"] = biasT
        in_maps.append(m)
    return in_maps


def _mask_mode(mask):
    m2 = np.asarray(mask)[0, 0]
    if (m2 == 1).all():
        return "none"
    if np.array_equal(m2 != 0, np.tril(np.ones(m2.shape, dtype=bool))):
        return "causal"
    return "generic"


def kernel(**inputs) -> np.ndarray:
    mode = _mask_mode(inputs["mask"])
    if mode not in _CACHE:
        _CACHE[mode] = _build(mode)
    nc = _CACHE[mode]
    in_maps = _prep(inputs, mode)
    res = bass_utils.run_bass_kernel_spmd(nc, in_maps,
                                          core_ids=list(range(N_CORES)))
    out = res.results[0]["out"].astype(np.float32)
    for c in range(1, N_CORES):
        out += res.results[c]["out"]
    return out.reshape(B, S, D)


if __name__ == "__main__":
    rng = np.random.default_rng(0)
    inputs = {
        "query": rng.standard_normal((B, S, D)).astype(np.float32),
        "key": rng.standard_normal((B, S, D)).astype(np.float32),
        "value": rng.standard_normal((B, S, D)).astype(np.float32),
        "mask": np.tril(np.ones((S, S), np.int32))[None, None],
        "Wq": (rng.standard_normal((D, D)) / 32).astype(np.float32),
        "Wk": (rng.standard_normal((D, D)) / 32).astype(np.float32),
        "Wv": (rng.standard_normal((D, D)) / 32).astype(np.float32),
        "Wo": (rng.standard_normal((D, D)) / 32).astype(np.float32),
    }
    got = kernel(**inputs)
    print("kernel ran, out shape", got.shape, "finite:", np.isfinite(got).all())
